# revision 1
# baseline (speedup 1.0000x reference)
"""Trainium2 Bass kernel for AllGNN message passing.

Computes, for full inputs:
    h   = x @ W_in + b_in
    deg = adj.sum(axis=1, keepdims=True)
    agg = (adj @ h) / (deg + 1)
    out = agg @ W_cls + b_cls

Key algebra: row scaling commutes with the right matmul, so
    out = (adj @ G)[:, :C] / (deg+1) + b_cls
with G = [x @ W2 + b2 | ones], W2 = W_in @ W_cls, b2 = b_in @ W_cls.
The ones column's product recovers deg. The ones column is folded into the
G build by padding W2 with a zero column and b2 with a one.

Sharding: row-shard adj over 8 cores; x is shipped pre-transposed (and
pre-cast to bf16) so each core computes the full G locally with W2 as the
stationary matmul operand — no collectives at all. Each core then streams
its adj row-block once from HBM (SWDGE fp32->bf16 cast; adj is 0/1 so bf16
is exact), transposes 128x128 blocks on the PE (is_transpose against
identity), and accumulates out.T = G.T @ adj.T chunk-by-chunk in PSUM with
G tiles as the stationary operand. Group 0's accumulation is deferred a
couple of chunks so G is ready before the first accumulating matmul.
"""

import numpy as np

import concourse.bass as bass
from concourse import bacc
import concourse.mybir as mybir
import concourse.tile as tile
from concourse.bass_utils import run_bass_kernel_spmd

import ml_dtypes

N_CORES = 8
N_NODES = 12000
IN_CH = 256
HID = 64
N_CLS = 40

JW = 128  # j (contraction) tile width
IW = 128  # i (output-row) tile width
XC = 512  # x/g chunk width (columns of g.T per matmul)


def _ceil_div(a, b):
    return -(-a // b)


def build_gnn(
    n_nodes=N_NODES,
    n_cores=N_CORES,
    in_ch=IN_CH,
    hid=HID,
    n_cls=N_CLS,
    stage_jtiles=12,
    group_its=4,
    strip_bufs=None,
    nat_bufs=None,
    act_copy_every=2,
    use_is_transpose=True,
    chain0_delay=4,
    g_per_chunk=4,
):
    f32 = mybir.dt.float32
    bf16 = mybir.dt.bfloat16
    mult = mybir.AluOpType.mult
    add = mybir.AluOpType.add

    assert n_nodes % n_cores == 0
    rows = n_nodes // n_cores
    assert in_ch % 128 == 0
    n_kt = in_ch // 128
    assert hid <= 128 and n_cls + 1 <= 128
    gc = n_cls + 1  # G columns: [g | ones]

    n_jt = _ceil_div(n_nodes, JW)
    n_it = _ceil_div(rows, IW)
    stage_cols = stage_jtiles * JW
    n_chunks = _ceil_div(n_nodes, stage_cols)
    n_xc = _ceil_div(n_nodes, XC)

    nc = bacc.Bacc(num_devices=n_cores)

    adj_h = nc.dram_tensor("adj_blk", [rows, n_nodes], f32, kind="ExternalInput")
    xt_h = nc.dram_tensor("x_Ti", [128, n_xc, n_kt, XC], bf16, kind="ExternalInput")
    win_h = nc.dram_tensor("W_in", [in_ch, hid], f32, kind="ExternalInput")
    bin_h = nc.dram_tensor("b_in", [hid], f32, kind="ExternalInput")
    wcls_h = nc.dram_tensor("W_cls", [hid, n_cls], f32, kind="ExternalInput")
    bcls_h = nc.dram_tensor("b_cls", [n_cls], f32, kind="ExternalInput")
    out_h = nc.dram_tensor("out_blk", [rows, n_cls], f32, kind="ExternalOutput")

    id_f_dram = nc.inline_tensor(np.eye(128, dtype=np.float32), name="ident_f32")
    id_b_dram = nc.inline_tensor(
        np.eye(128).astype(ml_dtypes.bfloat16), name="ident_bf16"
    )

    n_groups = _ceil_div(n_it, group_its)
    sbufs = (
        (min(n_chunks, chain0_delay + 1) + 2) * stage_jtiles + 8
        if strip_bufs is None
        else strip_bufs
    )
    nbufs = 5 * group_its if nat_bufs is None else nat_bufs
    group_w_max = min(group_its * IW, 512)
    assert group_its * IW <= 512
    pt_dt = bf16 if use_is_transpose else f32

    with tile.TileContext(nc) as tc:
        with (
            tc.tile_pool(name="singles", bufs=1) as singles,
            tc.tile_pool(name="nat", bufs=nbufs) as nat_pool,
            tc.tile_pool(name="strip", bufs=sbufs) as strip_pool,
            tc.tile_pool(name="outp", bufs=3) as out_pool,
        ):
            id_f = singles.tile([128, 128], f32, tag="id_f")
            nc.sync.dma_start(out=id_f, in_=id_f_dram[:])
            id_b = singles.tile([128, 128], bf16, tag="id_b")
            nc.sync.dma_start(out=id_b, in_=id_b_dram[:])
            # b_cls broadcast across partitions
            bcls_sb = singles.tile([128, n_cls], f32, tag="bcls")
            bc = bcls_h[:]
            nc.gpsimd.dma_start(
                out=bcls_sb,
                in_=bass.AP(tensor=bc.tensor, offset=bc.offset, ap=[[0, 128]] + bc.ap),
            )
            qn = _ceil_div(n_jt, 4)
            G_parts = [
                singles.tile([128, qn, gc], bf16, tag=f"G{q}", name=f"G{q}")
                for q in range(4)
            ]

            def G_at(jt):
                return G_parts[jt // qn], jt % qn

            # ---- Phase A: G = [x @ W2 + b2 | 1] computed fully per core ----
            with (
                tc.tile_pool(name="ph_a", bufs=3) as pa,
                tc.tile_pool(name="psc", bufs=1, space="PSUM") as psc,
            ):
                psb = psc
                win_sb = pa.tile([128, n_kt, hid], f32, tag="win")
                nc.sync.dma_start(
                    out=win_sb, in_=win_h[:].rearrange("(t p) h -> p t h", p=128)
                )
                wcls_sb = pa.tile([hid, n_cls], f32, tag="wcls")
                nc.sync.dma_start(out=wcls_sb, in_=wcls_h[:])
                bin_sb = pa.tile([hid, 1], f32, tag="bin")
                bi = bin_h[:]
                nc.sync.dma_start(
                    out=bin_sb,
                    in_=bass.AP(
                        tensor=bi.tensor, offset=bi.offset, ap=bi.ap + [[0, 1]]
                    ),
                )

                ones_sb = pa.tile([1, 128], f32, tag="ones")
                nc.vector.memset(ones_sb, 1.0)
                # G ones column (deg recovery), written once
                for gp in G_parts:
                    nc.vector.memset(gp[:, :, n_cls:gc], 1.0)

                # W_in.T tiles via PE transpose (fp32)
                winT_sb = pa.tile([hid, n_kt, 128], f32, tag="winT")
                for t in range(n_kt):
                    ps_w = psb.tile([128, 512], f32, tag="acc", name="acc", bufs=4)
                    ps = ps_w[:hid, :128]
                    nc.tensor.matmul(
                        ps, lhsT=win_sb[:, t, :], rhs=id_f, start=True, stop=True
                    )
                    nc.vector.tensor_copy(winT_sb[:, t, :], ps)
                # W2 = W_in @ W_cls -> bf16
                w2b_sb = pa.tile([128, n_kt, n_cls], bf16, tag="w2b")
                for t in range(n_kt):
                    ps_w = psb.tile([128, 512], f32, tag="acc", name="acc", bufs=4)
                    ps = ps_w[:, :n_cls]
                    nc.tensor.matmul(
                        ps, lhsT=winT_sb[:, t, :], rhs=wcls_sb, start=True, stop=True
                    )
                    nc.vector.tensor_copy(w2b_sb[:, t, :], ps)
                # b2 = b_in @ W_cls broadcast to [128, n_cls]
                ps_b2w = psb.tile([128, 512], f32, tag="acc", name="acc", bufs=4)
                ps_b2 = ps_b2w[:1, :n_cls]
                nc.tensor.matmul(ps_b2, lhsT=bin_sb, rhs=wcls_sb, start=True, stop=True)
                b2row = pa.tile([1, n_cls], f32, tag="b2row")
                nc.vector.tensor_copy(b2row, ps_b2)
                ps_b2bw = psb.tile([128, 512], f32, tag="acc", name="acc", bufs=4)
                ps_b2b = ps_b2bw[:, :n_cls]
                nc.tensor.matmul(ps_b2b, lhsT=ones_sb, rhs=b2row, start=True, stop=True)
                b2b_sb = pa.tile([128, n_cls], f32, tag="b2b")
                nc.vector.tensor_copy(b2b_sb, ps_b2b)

                # g = x @ W2 + b2, with pre-transposed x slices as the
                # stationary operand, written straight into G_sb node tiles.
                # Emitted as jobs interleaved into group 0's streaming chunks
                # so phase A's PE work doesn't starve the transpose pipeline.
                def g_job(ch):
                    c0 = ch * XC
                    cw = min(XC, n_nodes - c0)
                    xts = pa.tile([128, n_kt, XC], bf16, tag="xts", name="xts")
                    nc.sync.dma_start(out=xts, in_=xt_h[:, ch, :, :])
                    for q in range(_ceil_div(cw, JW)):
                        qw = min(JW, cw - q * JW)
                        jt = (c0 + q * JW) // JW
                        ps_gw = psb.tile(
                            [128, 512], f32, tag="acc", name="acc", bufs=4
                        )
                        ps_g = ps_gw[:, :n_cls]
                        for t in range(n_kt):
                            nc.tensor.matmul(
                                ps_g[:qw, :],
                                lhsT=xts[:, t, q * JW : q * JW + qw],
                                rhs=w2b_sb[:, t, :],
                                start=(t == 0),
                                stop=(t == n_kt - 1),
                            )
                        gp, gi = G_at(jt)
                        nc.vector.tensor_add(
                            gp[:qw, gi, 0:n_cls], ps_g[:qw, :], b2b_sb[:qw]
                        )

                g_jobs = list(range(n_xc))
                g_per = g_per_chunk

                # -- Phase B: stream adj, transpose on PE, accumulate out.T --
                psc = psb
                copy_state = [0]

                class AccChain:
                    """One group's out.T accumulation: even/odd j-tiles go to
                    two PSUM banks so consecutive matmuls pipeline."""

                    NWAY = 3

                    def __init__(self, grp_info):
                        self.grp_info = grp_info
                        self.next_jt = 0
                        self.ps = [
                            psc.tile(
                                [gc, group_w_max], f32, tag="acc", name="acc", bufs=4
                            )
                            for _ in range(self.NWAY)
                        ]

                    def feed(self, upto_jt):
                        its, widths, offs, gw, strips = self.grp_info
                        while self.next_jt < upto_jt:
                            jt = self.next_jt
                            jw = min(JW, n_nodes - jt * JW)
                            ps = self.ps[jt % self.NWAY]
                            gp, gi = G_at(jt)
                            nc.tensor.matmul(
                                ps[:, :gw],
                                lhsT=gp[:jw, gi, :],
                                rhs=strips[jt][:jw, :gw],
                                start=(jt < self.NWAY),
                                stop=(jt >= n_jt - self.NWAY),
                                skip_group_check=True,
                            )
                            self.next_jt += 1
                        if self.next_jt == n_jt:
                            self._finalize()
                            return True
                        return False

                    def _finalize(self):
                        its, widths, offs, gw, strips = self.grp_info
                        U_sb = out_pool.tile([gc, group_w_max], f32, tag="U")
                        nc.vector.tensor_copy(U_sb[:, :gw], self.ps[0][:, :gw])
                        for w in range(1, self.NWAY):
                            nc.vector.tensor_add(
                                U_sb[:, :gw], U_sb[:, :gw], self.ps[w][:, :gw]
                            )
                        for k, it in enumerate(its):
                            i0 = it * IW
                            p = widths[k]
                            go = offs[k]
                            ps_fw = psc.tile(
                                [128, 512], f32, tag="acc", name="acc", bufs=4
                            )
                            ps_f = ps_fw[:, :gc]
                            nc.tensor.matmul(
                                ps_f[:p, :],
                                lhsT=U_sb[:, go : go + p],
                                rhs=id_f[:gc, :gc],
                                start=True,
                                stop=True,
                                skip_group_check=True,
                            )
                            deg1 = out_pool.tile([128, 1], f32, tag="deg1")
                            nc.vector.tensor_scalar_add(
                                deg1[:p], ps_f[:p, n_cls:gc], 1.0
                            )
                            rcp = out_pool.tile([128, 1], f32, tag="rcp")
                            nc.vector.reciprocal(rcp[:p], deg1[:p])
                            o_sb = out_pool.tile([128, n_cls], f32, tag="o")
                            nc.vector.scalar_tensor_tensor(
                                out=o_sb[:p],
                                in0=ps_f[:p, 0:n_cls],
                                scalar=rcp[:p],
                                in1=bcls_sb[:p],
                                op0=mult,
                                op1=add,
                            )
                            nc.sync.dma_start(out=out_h[i0 : i0 + p, :], in_=o_sb[:p])

                def emit_chunk(grp_info, c):
                    its, widths, offs, gw, strips = grp_info
                    j0 = c * stage_cols
                    cw = min(stage_cols, n_nodes - j0)
                    nat_tiles = []
                    for k, it in enumerate(its):
                        i0 = it * IW
                        p = widths[k]
                        nt_ = nat_pool.tile([128, stage_cols], bf16, tag="nat")
                        nc.gpsimd.dma_start(
                            out=nt_[:p, :cw], in_=adj_h[i0 : i0 + p, j0 : j0 + cw]
                        )
                        nat_tiles.append(nt_)
                    jt_lo = c * stage_jtiles
                    jt_hi = min((c + 1) * stage_jtiles, n_jt)
                    for jt in range(jt_lo, jt_hi):
                        jw = min(JW, n_nodes - jt * JW)
                        off = jt * JW - j0
                        ps = psc.tile(
                            [128, group_w_max], pt_dt, tag="pt", name="pt", bufs=4
                        )
                        for k in range(len(its)):
                            p = widths[k]
                            go = offs[k]
                            if use_is_transpose:
                                nc.tensor.matmul(
                                    ps[:jw, go : go + p],
                                    lhsT=nat_tiles[k][:p, off : off + jw],
                                    rhs=id_b[:p, :p],
                                    is_transpose=True,
                                )
                            else:
                                nc.tensor.matmul(
                                    ps[:jw, go : go + p],
                                    lhsT=nat_tiles[k][:p, off : off + jw],
                                    rhs=id_b[:p, :p],
                                    start=True,
                                    stop=True,
                                )
                        st = strips[jt]
                        if act_copy_every and copy_state[0] % act_copy_every == (
                            act_copy_every - 1
                        ):
                            nc.scalar.copy(st[:jw, :gw], ps[:jw, :gw])
                        else:
                            nc.vector.tensor_copy(st[:jw, :gw], ps[:jw, :gw])
                        copy_state[0] += 1
                    return jt_hi

                # every group accumulates its own strips within each chunk;
                # group 0 defers by `chain0_delay` chunks so G (phase A) is
                # ready before the first accumulating matmul.
                for grp in range(n_groups):
                    its = list(range(grp * group_its, min((grp + 1) * group_its, n_it)))
                    widths = [min(IW, rows - it * IW) for it in its]
                    offs = [sum(widths[:k]) for k in range(len(its))]
                    gw = sum(widths)
                    strips = [
                        strip_pool.tile(
                            [128, group_w_max], bf16, tag="strip", name="strip"
                        )
                        for _ in range(n_jt)
                    ]
                    grp_info = (its, widths, offs, gw, strips)
                    chain = None
                    delay = chain0_delay if grp == 0 else 0
                    for c in range(n_chunks):
                        jt_hi = emit_chunk(grp_info, c)
                        # interleave phase A's g computation into group 0's
                        # first chunks (keeps the PE fed without a long
                        # serial phase A before streaming starts)
                        for _ in range(g_per):
                            if g_jobs:
                                g_job(g_jobs.pop(0))
                        if chain is None and c >= delay:
                            chain = AccChain(grp_info)
                        if chain is not None:
                            chain.feed(jt_hi)

    nc.compile()
    return nc


_CACHE = {}


def _get_nc():
    if "nc" not in _CACHE:
        _CACHE["nc"] = build_gnn()
    return _CACHE["nc"]


def make_in_maps(x, adj, W_in, b_in, W_cls, b_cls):
    rows = adj.shape[0] // N_CORES
    n = x.shape[0]
    n_kt = x.shape[1] // 128
    n_xc = _ceil_div(n, XC)
    xp = np.zeros((x.shape[1], n_xc * XC), dtype=np.float32)
    xp[:, :n] = np.asarray(x, dtype=np.float32).T
    x_Ti = np.ascontiguousarray(
        xp.reshape(n_kt, 128, n_xc, XC).transpose(1, 2, 0, 3)
    ).astype(ml_dtypes.bfloat16)
    in_maps = []
    for c in range(N_CORES):
        sl = slice(c * rows, (c + 1) * rows)
        in_maps.append(
            {
                "adj_blk": np.ascontiguousarray(adj[sl]),
                "x_Ti": x_Ti,
                "W_in": W_in,
                "b_in": b_in,
                "W_cls": W_cls,
                "b_cls": b_cls,
            }
        )
    return in_maps


def kernel(x, adj, W_in, b_in, W_cls, b_cls):
    x = np.asarray(x, dtype=np.float32)
    adj = np.asarray(adj, dtype=np.float32)
    W_in = np.asarray(W_in, dtype=np.float32)
    b_in = np.asarray(b_in, dtype=np.float32)
    W_cls = np.asarray(W_cls, dtype=np.float32)
    b_cls = np.asarray(b_cls, dtype=np.float32)

    nc = _get_nc()
    in_maps = make_in_maps(x, adj, W_in, b_in, W_cls, b_cls)
    res = run_bass_kernel_spmd(nc, in_maps, core_ids=list(range(N_CORES)))
    outs = [res.results[c]["out_blk"] for c in range(N_CORES)]
    return np.concatenate(outs, axis=0).astype(np.float32)



# revision 3
# speedup vs baseline: 2.0418x; 2.0418x over previous
"""Trainium2 Bass kernel for AllGNN message passing.

Computes, for full inputs:
    h   = x @ W_in + b_in
    deg = adj.sum(axis=1, keepdims=True)
    agg = (adj @ h) / (deg + 1)
    out = agg @ W_cls + b_cls

Key algebra: row scaling commutes with the right matmul, so
    out = (adj @ G)[:, :C] / (deg+1) + b_cls
with G = [x @ W2 + b2 | ones], W2 = W_in @ W_cls, b2 = b_in @ W_cls.
The ones column's product recovers deg.

Sharding: row-shard adj over 8 cores. The adj row-block is shipped
pre-transposed (adjT = adj_blk.T, [N, rows]) and pre-cast to fp8e4 on the
host -- adj is 0/1 so fp8 is exact and HBM traffic drops 4x vs fp32, and
no on-device transpose is needed at all. x is shipped pre-transposed in
bf16 (replicated) so each core computes the full G locally; no collectives.

Main loop: for each 128-row strip of adjT (fp8, streamed once from HBM),
accumulate out.T[c, i] += G[j, c] * adjT[j, i] with the 41-col G tile as
the stationary operand and the fp8 strip as the moving operand (mixed
bf16 x fp8 matmul, fp32 PSUM accumulation). Three persistent PSUM banks
hold out.T chunks [41, 512/512/480]. G production (x @ W2) is interleaved
one chunk ahead of consumption so the PE never waits on phase A.
"""

import numpy as np

import concourse.bass as bass
from concourse import bacc
import concourse.mybir as mybir
import concourse.tile as tile
from concourse.bass_utils import run_bass_kernel_spmd

import ml_dtypes

N_CORES = 8
N_NODES = 12000
IN_CH = 256
HID = 64
N_CLS = 40

ROWS = N_NODES // N_CORES        # 1500 output rows per core
ROWS_PAD = 1504                  # padded i-dim (8B-aligned fp8 lines)
JW = 128                         # j (contraction) tile width
N_JT = -(-N_NODES // JW)         # 94 j-tiles
N_KT = IN_CH // 128              # 2 k-tiles for x @ W2
GC = N_CLS + 1                   # G columns: [g | ones]
GJT = 8                          # j-tiles per G-production chunk
N_GCH = -(-N_JT // GJT)          # 12 G chunks
JPAD = N_GCH * GJT * JW          # 12288 padded j-dim for x
# out.T chunk layout across the padded i-dim: 3 PSUM banks
PSU_CHUNKS = [(0, 512), (512, 512), (1024, ROWS_PAD - 1024)]


def build_gnn(
    n_cores=N_CORES,
    strip_bufs=16,
    x_bufs=3,
    g_lookahead=1,
):
    f32 = mybir.dt.float32
    bf16 = mybir.dt.bfloat16
    f8 = mybir.dt.float8e4
    mult = mybir.AluOpType.mult
    add = mybir.AluOpType.add

    nc = bacc.Bacc(num_devices=n_cores)

    adjT_h = nc.dram_tensor("adjT", [N_NODES, ROWS_PAD], f8, kind="ExternalInput")
    xt_h = nc.dram_tensor("x_Ti", [128, N_KT, JPAD], bf16, kind="ExternalInput")
    win_h = nc.dram_tensor("W_in", [IN_CH, HID], f32, kind="ExternalInput")
    bin_h = nc.dram_tensor("b_in", [HID], f32, kind="ExternalInput")
    wcls_h = nc.dram_tensor("W_cls", [HID, N_CLS], f32, kind="ExternalInput")
    bcls_h = nc.dram_tensor("b_cls", [N_CLS], f32, kind="ExternalInput")
    out_h = nc.dram_tensor("out_blk", [ROWS, N_CLS], f32, kind="ExternalOutput")

    id_f_dram = nc.inline_tensor(np.eye(128, dtype=np.float32), name="ident_f32")

    with tile.TileContext(nc) as tc:
        with (
            tc.tile_pool(name="singles", bufs=1) as singles,
            tc.tile_pool(name="gpool", bufs=N_GCH) as g_pool,
            tc.tile_pool(name="xpool", bufs=x_bufs) as x_pool,
            tc.tile_pool(name="spool", bufs=strip_bufs) as strip_pool,
            tc.tile_pool(name="opool", bufs=4) as out_pool,
            tc.tile_pool(name="psum", bufs=1, space="PSUM") as psum_pool,
        ):
            id_f = singles.tile([128, 128], f32, tag="id_f")
            nc.sync.dma_start(out=id_f, in_=id_f_dram[:])
            # b_cls broadcast across partitions
            bcls_sb = singles.tile([128, N_CLS], f32, tag="bcls")
            bc = bcls_h[:]
            nc.gpsimd.dma_start(
                out=bcls_sb,
                in_=bass.AP(tensor=bc.tensor, offset=bc.offset, ap=[[0, 128]] + bc.ap),
            )

            # persistent PSUM banks for the out.T accumulation
            psU = [
                psum_pool.tile([128, 512], f32, tag=f"U{i}", name=f"U{i}", bufs=1)
                for i in range(len(PSU_CHUNKS))
            ]

            # ---- Phase A: W2 = W_in @ W_cls, b2 = b_in @ W_cls (tiny) ----
            win_sb = singles.tile([128, N_KT, HID], f32, tag="win")
            nc.sync.dma_start(
                out=win_sb, in_=win_h[:].rearrange("(t p) h -> p t h", p=128)
            )
            wcls_sb = singles.tile([HID, N_CLS], f32, tag="wcls")
            nc.sync.dma_start(out=wcls_sb, in_=wcls_h[:])
            bin_sb = singles.tile([HID, 1], f32, tag="bin")
            bi = bin_h[:]
            nc.sync.dma_start(
                out=bin_sb,
                in_=bass.AP(tensor=bi.tensor, offset=bi.offset, ap=bi.ap + [[0, 1]]),
            )
            ones_sb = singles.tile([1, 128], f32, tag="ones")
            nc.vector.memset(ones_sb, 1.0)

            # W_in.T tiles via PE transpose (fp32)
            winT_sb = singles.tile([HID, N_KT, 128], f32, tag="winT")
            for t in range(N_KT):
                ps_w = psum_pool.tile([128, 512], f32, tag="g", bufs=4)
                ps = ps_w[:HID, :128]
                nc.tensor.matmul(
                    ps, lhsT=win_sb[:, t, :], rhs=id_f, start=True, stop=True
                )
                nc.vector.tensor_copy(winT_sb[:, t, :], ps)
            # W2 = W_in @ W_cls -> bf16
            w2b_sb = singles.tile([128, N_KT, N_CLS], bf16, tag="w2b")
            for t in range(N_KT):
                ps_w = psum_pool.tile([128, 512], f32, tag="g", bufs=4)
                ps = ps_w[:, :N_CLS]
                nc.tensor.matmul(
                    ps, lhsT=winT_sb[:, t, :], rhs=wcls_sb, start=True, stop=True
                )
                nc.vector.tensor_copy(w2b_sb[:, t, :], ps)
            # b2 = b_in @ W_cls broadcast to [128, N_CLS]
            ps_b2w = psum_pool.tile([128, 512], f32, tag="g", bufs=4)
            ps_b2 = ps_b2w[:1, :N_CLS]
            nc.tensor.matmul(ps_b2, lhsT=bin_sb, rhs=wcls_sb, start=True, stop=True)
            b2row = singles.tile([1, N_CLS], f32, tag="b2row")
            nc.vector.tensor_copy(b2row, ps_b2)
            ps_b2bw = psum_pool.tile([128, 512], f32, tag="g", bufs=4)
            ps_b2b = ps_b2bw[:, :N_CLS]
            nc.tensor.matmul(ps_b2b, lhsT=ones_sb, rhs=b2row, start=True, stop=True)
            b2b_sb = singles.tile([128, N_CLS], f32, tag="b2b")
            nc.vector.tensor_copy(b2b_sb, ps_b2b)

            # ---- G production: one chunk = GJT j-tiles of G = x @ W2 + b2 ----
            G_tiles = {}

            def g_job(q):
                if q >= N_GCH or q in G_tiles:
                    return
                xts = x_pool.tile([128, N_KT, GJT * JW], bf16, tag="xts")
                nc.scalar.dma_start(
                    out=xts, in_=xt_h[:, :, q * GJT * JW : (q + 1) * GJT * JW]
                )
                gt = g_pool.tile([128, GJT, GC], bf16, tag="G")
                nc.vector.memset(gt[:, :, N_CLS:GC], 1.0)
                for s in range(GJT):
                    ps_gw = psum_pool.tile([128, 512], f32, tag="g", bufs=4)
                    ps_g = ps_gw[:, :N_CLS]
                    for t in range(N_KT):
                        nc.tensor.matmul(
                            ps_g,
                            lhsT=xts[:, t, s * JW : (s + 1) * JW],
                            rhs=w2b_sb[:, t, :],
                            start=(t == 0),
                            stop=(t == N_KT - 1),
                        )
                    nc.vector.tensor_add(gt[:, s, 0:N_CLS], ps_g, b2b_sb)
                G_tiles[q] = gt

            # ---- Phase B: stream adjT strips, accumulate out.T ----
            g_job(0)
            for jt in range(N_JT):
                if jt % GJT == 0:
                    for la in range(1, g_lookahead + 1):
                        g_job(jt // GJT + la)
                jw = min(JW, N_NODES - jt * JW)
                strip = strip_pool.tile([128, ROWS_PAD], f8, tag="strip")
                nc.sync.dma_start(
                    out=strip[:jw], in_=adjT_h[jt * JW : jt * JW + jw, :]
                )
                gt = G_tiles[jt // GJT]
                s = jt % GJT
                for ch, (c0, cw) in enumerate(PSU_CHUNKS):
                    nc.tensor.matmul(
                        psU[ch][:GC, :cw],
                        lhsT=gt[:jw, s, :],
                        rhs=strip[:jw, c0 : c0 + cw],
                        start=(jt == 0),
                        stop=(jt == N_JT - 1),
                        skip_group_check=True,
                    )

            # ---- Finalize: transpose back, divide by deg+1, add bias ----
            for ch, (c0, cw) in enumerate(PSU_CHUNKS):
                U_sb = out_pool.tile([GC, 512], f32, tag="U")
                nc.vector.tensor_copy(U_sb[:, :cw], psU[ch][:GC, :cw])
                for k in range(-(-cw // 128)):
                    i0 = c0 + k * 128
                    if i0 >= ROWS:
                        break
                    p = min(128, ROWS - i0)
                    pw = min(128, cw - k * 128)
                    p = min(p, pw)
                    ps_fw = psum_pool.tile([128, 512], f32, tag="g", bufs=4)
                    ps_f = ps_fw[:, :GC]
                    nc.tensor.matmul(
                        ps_f[:pw, :],
                        lhsT=U_sb[:, k * 128 : k * 128 + pw],
                        rhs=id_f[:GC, :GC],
                        start=True,
                        stop=True,
                        skip_group_check=True,
                    )
                    deg1 = out_pool.tile([128, 1], f32, tag="deg1")
                    nc.vector.tensor_scalar_add(deg1[:p], ps_f[:p, N_CLS:GC], 1.0)
                    rcp = out_pool.tile([128, 1], f32, tag="rcp")
                    nc.vector.reciprocal(rcp[:p], deg1[:p])
                    o_sb = out_pool.tile([128, N_CLS], f32, tag="o")
                    nc.vector.scalar_tensor_tensor(
                        out=o_sb[:p],
                        in0=ps_f[:p, 0:N_CLS],
                        scalar=rcp[:p],
                        in1=bcls_sb[:p],
                        op0=mult,
                        op1=add,
                    )
                    nc.sync.dma_start(out=out_h[i0 : i0 + p, :], in_=o_sb[:p])

    nc.compile()
    return nc


_CACHE = {}


def _get_nc():
    if "nc" not in _CACHE:
        _CACHE["nc"] = build_gnn()
    return _CACHE["nc"]


def make_in_maps(x, adj, W_in, b_in, W_cls, b_cls):
    f8 = ml_dtypes.float8_e4m3
    adj8 = np.asarray(adj, dtype=np.float32).astype(f8)
    xp = np.zeros((IN_CH, JPAD), dtype=np.float32)
    xp[:, :N_NODES] = np.asarray(x, dtype=np.float32).T
    x_Ti = np.ascontiguousarray(
        xp.reshape(N_KT, 128, JPAD).transpose(1, 0, 2)
    ).astype(ml_dtypes.bfloat16)
    in_maps = []
    for c in range(N_CORES):
        sl = slice(c * ROWS, (c + 1) * ROWS)
        blk = np.zeros((N_NODES, ROWS_PAD), dtype=f8)
        blk[:, :ROWS] = adj8[sl, :].T
        in_maps.append(
            {
                "adjT": blk,
                "x_Ti": x_Ti,
                "W_in": W_in,
                "b_in": b_in,
                "W_cls": W_cls,
                "b_cls": b_cls,
            }
        )
    return in_maps


def kernel(x, adj, W_in, b_in, W_cls, b_cls):
    x = np.asarray(x, dtype=np.float32)
    adj = np.asarray(adj, dtype=np.float32)
    W_in = np.asarray(W_in, dtype=np.float32)
    b_in = np.asarray(b_in, dtype=np.float32)
    W_cls = np.asarray(W_cls, dtype=np.float32)
    b_cls = np.asarray(b_cls, dtype=np.float32)

    nc = _get_nc()
    in_maps = make_in_maps(x, adj, W_in, b_in, W_cls, b_cls)
    res = run_bass_kernel_spmd(nc, in_maps, core_ids=list(range(N_CORES)))
    outs = [res.results[c]["out_blk"] for c in range(N_CORES)]
    return np.concatenate(outs, axis=0).astype(np.float32)
